# revision 19
# baseline (speedup 1.0000x reference)
"""Trainium2 Bass kernel for a GPT-2 style transformer block (pre-LN, no mask).

Reference shapes: x [B=2, T=2048, C=1024], H=16 heads, MLP hidden 4C=4096.

Sharding (8 NeuronCores): data-parallel over B (cores 0-3 -> batch 0,
cores 4-7 -> batch 1); within each 4-core group the 2048 query rows are
split 512 per core. Every core redundantly computes K and V for its full
batch from a replicated (rotated) copy of x, so no collectives are needed:
attention rows and the MLP are fully local to a core. The per-core x is
rotated so that the core's own 512 query rows always sit at rows 0:512,
keeping the SPMD program identical across cores (softmax over the key
axis is permutation-invariant, so rotating the key order is harmless).

Compute layout: activations feeding matmul contractions are kept
feature-major ("transposed", [C, t]) via the DMA xbar transpose; scores
are computed as S^T = K Q^T per head ([tk, tq]) with two heads packed
into the 128-wide contraction via row tiling; exp runs on the scalar
engine straight out of PSUM; P @ V uses a [V | ones] stationary operand
so the softmax denominators accumulate in the same PSUM tile as Y^T.

Emission is interleaved for engine overlap: LayerNorm tiles alternate
with V matmuls; each head-pair's K^T/Q^T matmuls are followed
immediately by that pair's attention so the scalar engine's exp stream
overlaps the tensor engine's QKV work and the PE never idles long
enough for the HAM clock gate to re-throttle.

Weights are pre-tiled on the host into the exact SBUF layouts so every
DMA is a contiguous 128-partition transfer.
"""

import numpy as np
import ml_dtypes

import concourse.bass as bass
import concourse.bacc as bacc
import concourse.tile as tile
from concourse import mybir
from concourse.bass import ts, ds
from concourse.bass_utils import run_bass_kernel_spmd

f32 = mybir.dt.float32
bf16 = mybir.dt.bfloat16
AF = mybir.ActivationFunctionType
OP = mybir.AluOpType

B, T, C, H = 2, 2048, 1024, 16
DH = C // H          # 64
F = 4 * C            # 4096
NCORES = 8
GROUP = 4            # cores per batch
TQ = T // GROUP      # 512 query rows per core
NT = T // 128        # 16 token tiles
CCH = C // 128       # 8 contraction chunks over C
PAIRS = H // 2       # 8 head pairs
FT = F // 128        # 32 hidden tiles
QT = TQ // 128       # 4 own-row tiles

_CACHED = {}


def _bcast(ap, parts=128):
    """DRAM AP for a 1-D tensor broadcast across `parts` partitions."""
    return bass.AP(tensor=ap.tensor, offset=ap.offset, ap=[[0, parts]] + list(ap.ap))


def _build_program(trivial_ln1, trivial_ln2, trivial_b):
    nc = bacc.Bacc("TRN2", target_bir_lowering=False, debug=False,
                   num_devices=NCORES)

    xf = nc.dram_tensor("xf", [T, C], f32, kind="ExternalInput")
    # pre-tiled weights: [128 (c within chunk), CCH, out-features]
    wq = nc.dram_tensor("wq", [128, CCH, C], bf16, kind="ExternalInput")
    wk = nc.dram_tensor("wk", [128, CCH, C], bf16, kind="ExternalInput")
    wv = nc.dram_tensor("wv", [128, CCH, C], bf16, kind="ExternalInput")
    bqv = nc.dram_tensor("bq", [128, PAIRS], f32, kind="ExternalInput")
    bkv = nc.dram_tensor("bk", [128, PAIRS], f32, kind="ExternalInput")
    bvv = nc.dram_tensor("bv", [C], f32, kind="ExternalInput")
    ln1w = nc.dram_tensor("ln1w", [C], f32, kind="ExternalInput")
    ln1b = nc.dram_tensor("ln1b", [C], f32, kind="ExternalInput")
    ln2w = nc.dram_tensor("ln2w", [C], f32, kind="ExternalInput")
    ln2b = nc.dram_tensor("ln2b", [C], f32, kind="ExternalInput")
    wp = nc.dram_tensor("wp", [128, CCH, C], bf16, kind="ExternalInput")
    bp = nc.dram_tensor("bp", [C], f32, kind="ExternalInput")
    # wf pre-tiled per f'-tile: [FT, 128 (c), CCH, 128 (f')]
    wf = nc.dram_tensor("wf", [FT, 128, CCH, 128], bf16, kind="ExternalInput")
    bf_ = nc.dram_tensor("bf", [128, FT], f32, kind="ExternalInput")
    wm = nc.dram_tensor("wm", [F, C], bf16, kind="ExternalInput")
    bm = nc.dram_tensor("bm", [C], f32, kind="ExternalInput")
    out = nc.dram_tensor("out", [TQ, C], f32, kind="ExternalOutput")

    with tile.TileContext(nc) as tc:
        _emit(nc, tc, trivial_ln1, trivial_ln2, trivial_b,
              xf, wq, wk, wv, bqv, bkv, bvv, ln1w, ln1b, ln2w, ln2b,
              wp, bp, wf, bf_, wm, bm, out)
    nc.compile()
    return nc


def _emit(nc, tc, trivial_ln1, trivial_ln2, trivial_b,
          xf, wq, wk, wv, bqv, bkv, bvv, ln1w, ln1b, ln2w, ln2b,
          wp, bp, wf, bf_, wm, bm, out):
    from contextlib import ExitStack

    with ExitStack() as st:
        persist = st.enter_context(tc.tile_pool(name="persist", bufs=1))
        stat = st.enter_context(tc.tile_pool(name="stat", bufs=3))
        stream = st.enter_context(tc.tile_pool(name="stream", bufs=3))

        eps_t = persist.tile([128, 1], f32)
        nc.vector.memset(eps_t, 1e-5)

        def layer_norm(x_t, w_bc, b_bc, out_ap, trivial):
            """x_t [128, C] f32 -> out_ap [128, C] bf16 (normalized + affine)."""
            stats = stat.tile([128, 2, nc.vector.BN_STATS_DIM], f32, name="stats")
            nc.vector.bn_stats(out=stats[:, 0, :], in_=x_t[:, 0:512])
            nc.vector.bn_stats(out=stats[:, 1, :], in_=x_t[:, 512:1024])
            mv = stat.tile([128, nc.vector.BN_AGGR_DIM], f32, name="mv")
            nc.vector.bn_aggr(out=mv, in_=stats)
            rstd = stat.tile([128, 1], f32, name="rstd")
            nc.scalar.activation(rstd, mv[:, 1:2], AF.Sqrt, bias=eps_t)
            nc.vector.reciprocal(rstd, rstd)
            if trivial:
                nc.vector.tensor_scalar(out=out_ap, in0=x_t, scalar1=mv[:, 0:1],
                                        scalar2=rstd, op0=OP.subtract, op1=OP.mult)
            else:
                t1 = stat.tile([128, C], f32, name="t1", tag="ln_t1")
                nc.vector.tensor_scalar(out=t1, in0=x_t, scalar1=mv[:, 0:1],
                                        scalar2=rstd, op0=OP.subtract, op1=OP.mult)
                nc.vector.tensor_mul(t1, t1, w_bc)
                nc.vector.tensor_add(out_ap, t1, b_bc)

        # ---------------- pools (stack discipline per side) ----------------
        # left: pA = QKV-era (hT, wq, wk, ln1, biases); later pD = MLP-era
        # right: pR = v/ynT/wp/ln2 (through attn_proj); pB = wv (LN era only)
        stA = st.enter_context(ExitStack())
        pA = stA.enter_context(tc.tile_pool(name="pA", bufs=1, side="left"))
        pR = st.enter_context(tc.tile_pool(name="pR", bufs=1, side="right"))
        stB = st.enter_context(ExitStack())
        pB = stB.enter_context(tc.tile_pool(name="pB", bufs=1, side="right"))

        # wv + x stream first (the LN/V phase needs them immediately);
        # the q/k weights are emitted mid-loop so their DMA doesn't delay x
        wv_sb = pB.tile([128, CCH, C], bf16)
        nc.sync.dma_start(out=wv_sb, in_=wv.ap())
        if not trivial_b:
            bv_bc = pA.tile([128, C], f32)
            nc.sync.dma_start(out=bv_bc, in_=_bcast(bvv.ap()))
        else:
            bv_bc = None
        if not trivial_ln1:
            ln1w_bc = pA.tile([128, C], f32)
            nc.sync.dma_start(out=ln1w_bc, in_=_bcast(ln1w.ap()))
            ln1b_bc = pA.tile([128, C], f32)
            nc.sync.dma_start(out=ln1b_bc, in_=_bcast(ln1b.ap()))
        else:
            ln1w_bc = ln1b_bc = None

        hT = pA.tile([128, NT, CCH, 128], bf16)
        v_sb = pR.tile([128, NT, H, DH + 1], bf16)
        ynT = pR.tile([128, PAIRS, TQ], bf16)
        nc.vector.memset(v_sb[:, :, :, DH:DH + 1], 1.0)

        # ---- LN1 tiles interleaved with V matmuls ----
        with tc.tile_pool(name="v_ps", bufs=4, space="PSUM") as v_ps:
            for i in range(NT):
                x_t = stream.tile([128, C], f32, name="x_t", tag="x_t")
                nc.scalar.dma_start(out=x_t, in_=xf.ap()[ts(i, 128), :])
                h_t = stream.tile([128, C], bf16, name="h_t", tag="h_t")
                layer_norm(x_t, ln1w_bc, ln1b_bc, h_t, trivial_ln1)
                nc.sync.dma_start_transpose(hT[:, i], h_t[:])
                if i == 2:
                    # q/k weights arrive while the LN/V pipeline runs
                    wq_sb = pA.tile([128, CCH, C], bf16)
                    nc.sync.dma_start(out=wq_sb, in_=wq.ap())
                    wk_sb = pA.tile([128, CCH, C], bf16)
                    nc.sync.dma_start(out=wk_sb, in_=wk.ap())
                    bq_sb = pA.tile([128, PAIRS], f32)
                    nc.sync.dma_start(out=bq_sb, in_=bqv.ap())
                    bk_sb = pA.tile([128, PAIRS], f32)
                    nc.sync.dma_start(out=bk_sb, in_=bkv.ap())
                ps0 = v_ps.tile([128, 512], f32, name="ps_v0", tag="ps_v")
                ps1 = v_ps.tile([128, 512], f32, name="ps_v1", tag="ps_v")
                pss = (ps0, ps1)
                for c in range(CCH):
                    for n in range(C // 512):
                        nc.tensor.matmul(pss[n], hT[:, i, c, :],
                                         wv_sb[:, c, ds(512 * n, 512)],
                                         start=(c == 0), stop=(c == CCH - 1))
                for n in range(C // 512):
                    if trivial_b:
                        nc.scalar.activation(v_sb[:, i, 8 * n:8 * n + 8, 0:DH],
                                             pss[n], AF.Identity)
                    else:
                        nc.vector.tensor_add(v_sb[:, i, 8 * n:8 * n + 8, 0:DH],
                                             pss[n], bv_bc[:, ds(512 * n, 512)])
        stB.close()

        # wp prefetch during attention (DMA is idle there)
        wp_sb = pR.tile([128, CCH, C], bf16)
        nc.sync.dma_start(out=wp_sb, in_=wp.ap())
        if not trivial_ln2:
            ln2w_bc = pR.tile([128, C], f32)
            nc.sync.dma_start(out=ln2w_bc, in_=_bcast(ln2w.ap()))
            ln2b_bc = pR.tile([128, C], f32)
            nc.sync.dma_start(out=ln2b_bc, in_=_bcast(ln2b.ap()))
        else:
            ln2w_bc = ln2b_bc = None
        bp_bc = pR.tile([128, C], f32)
        nc.sync.dma_start(out=bp_bc, in_=_bcast(bp.ap()))

        # ---- per-pair K^T/Q^T + attention, interleaved ----
        scale = 1.0 / float(np.sqrt(DH))
        with tc.tile_pool(name="kq_ps", bufs=2, space="PSUM") as kq_ps, \
             tc.tile_pool(name="s_ps", bufs=2, space="PSUM") as s_ps, \
             tc.tile_pool(name="y_ps", bufs=1, space="PSUM") as y_ps, \
             tc.tile_pool(name="kq_sb", bufs=2) as kq_sb, \
             tc.tile_pool(name="att_sb", bufs=3) as att_sb:
            for j in range(PAIRS):
                kT_j = kq_sb.tile([128, T], bf16, name="kT_j", tag="kT_j")
                for n in range(T // 512):
                    ps = kq_ps.tile([128, 512], f32, name="ps_k", tag="ps_kq")
                    for c in range(CCH):
                        nc.tensor.matmul(ps, wk_sb[:, c, ts(j, 128)],
                                         hT[:, 4 * n:4 * n + 4, c, :],
                                         start=(c == 0), stop=(c == CCH - 1))
                    nc.vector.tensor_scalar(out=kT_j[:, ds(512 * n, 512)],
                                            in0=ps, scalar1=bk_sb[:, j:j + 1],
                                            scalar2=None, op0=OP.add)
                qT_j = kq_sb.tile([128, TQ], bf16, name="qT_j", tag="qT_j")
                ps = kq_ps.tile([128, 512], f32, name="ps_q", tag="ps_kq")
                for c in range(CCH):
                    nc.tensor.matmul(ps, wq_sb[:, c, ts(j, 128)],
                                     hT[:, 0:QT, c, :],
                                     start=(c == 0), stop=(c == CCH - 1))
                nc.vector.tensor_scalar(out=qT_j, in0=ps,
                                        scalar1=bq_sb[:, j:j + 1],
                                        scalar2=None, op0=OP.add)

                ps_y1 = y_ps.tile([DH + 1, 512], f32, name="ps_y1", tag="ps_y1")
                ps_y2 = y_ps.tile([DH + 1, 512], f32, name="ps_y2", tag="ps_y2")
                for cidx in range(NT):
                    ps_s = s_ps.tile([128, 1024], f32, name="ps_s", tag="ps_s")
                    nc.tensor.matmul(ps_s[:, 0:512],
                                     kT_j[0:64, ts(cidx, 128)],
                                     qT_j[0:64, :], start=True, stop=True)
                    nc.tensor.matmul(ps_s[:, 512:1024],
                                     kT_j[64:128, ts(cidx, 128)],
                                     qT_j[64:128, :], start=True, stop=True,
                                     tile_position=(64, 0))
                    pT = att_sb.tile([128, 1024], bf16, name="pT", tag="pT")
                    nc.scalar.activation(pT, ps_s, AF.Exp, scale=scale)
                    nc.tensor.matmul(ps_y1, v_sb[:, cidx, 2 * j, :],
                                     pT[:, 0:512],
                                     start=(cidx == 0), stop=(cidx == NT - 1))
                    nc.tensor.matmul(ps_y2, v_sb[:, cidx, 2 * j + 1, :],
                                     pT[:, 512:1024],
                                     start=(cidx == 0), stop=(cidx == NT - 1))
                for u, ps_y in ((0, ps_y1), (1, ps_y2)):
                    # custom-DVE ops mis-read PSUM at a partition offset, so
                    # stage the sums row to SBUF partition 0 with a plain copy
                    rs0 = att_sb.tile([1, 512], f32, name="rs0", tag="rs0")
                    nc.vector.tensor_copy(rs0, ps_y[DH:DH + 1, :])
                    rs = att_sb.tile([1, 512], f32, name="rs", tag="rs")
                    nc.vector.reciprocal_approx_fast(rs, rs0)
                    bc = att_sb.tile([64, 512], f32, name="bc", tag="bc")
                    nc.gpsimd.partition_broadcast(bc, rs)
                    nc.vector.tensor_mul(ynT[64 * u:64 * u + 64, j, :],
                                         ps_y[0:DH, :], bc)
        stA.close()

        # ---- attn projection + residual + LN2 + h2^T ----
        pD = st.enter_context(tc.tile_pool(name="pD", bufs=1, side="left"))
        x2 = pD.tile([128, QT, C], f32)
        h2T = pD.tile([128, QT, CCH, 128], bf16)
        bfc_sb = pD.tile([128, FT], f32)
        nc.sync.dma_start(out=bfc_sb, in_=bf_.ap())
        bm_bc = pD.tile([128, C], f32)
        nc.sync.dma_start(out=bm_bc, in_=_bcast(bm.ap()))

        with tc.tile_pool(name="ap_ps", bufs=2, space="PSUM") as ap_ps:
            for i in range(QT):
                xb_t = stream.tile([128, C], f32, name="xb_t", tag="x_t")
                nc.scalar.dma_start(out=xb_t, in_=xf.ap()[ts(i, 128), :])
                nc.vector.tensor_add(xb_t, xb_t, bp_bc)
                for n in range(C // 512):
                    ps = ap_ps.tile([128, 512], f32, name="ps_a", tag="ps_a")
                    for j in range(PAIRS):
                        nc.tensor.matmul(ps, ynT[:, j, ts(i, 128)],
                                         wp_sb[:, j, ds(512 * n, 512)],
                                         start=(j == 0), stop=(j == PAIRS - 1))
                    nc.vector.tensor_add(x2[:, i, ds(512 * n, 512)], ps,
                                         xb_t[:, ds(512 * n, 512)])
                h2_t = stream.tile([128, C], bf16, name="h2_t", tag="h_t")
                layer_norm(x2[:, i, :], ln2w_bc, ln2b_bc, h2_t, trivial_ln2)
                nc.sync.dma_start_transpose(h2T[:, i], h2_t[:])

        # ---- MLP ----
        gT = pD.tile([128, FT, TQ], bf16)
        with tc.tile_pool(name="fc_ps", bufs=4, space="PSUM") as fc_ps, \
             tc.tile_pool(name="wf_sb", bufs=4) as wf_pool:
            for t in range(FT):
                wf_t = wf_pool.tile([128, CCH, 128], bf16, name="wf_t", tag="wf_t")
                nc.sync.dma_start(out=wf_t, in_=wf.ap()[t])
                ps = fc_ps.tile([128, 512], f32, name="ps_f", tag="ps_f")
                for c in range(CCH):
                    nc.tensor.matmul(ps, wf_t[:, c, :], h2T[:, 0:QT, c, :],
                                     start=(c == 0), stop=(c == CCH - 1))
                nc.scalar.activation(gT[:, t, :], ps, AF.Gelu_apprx_tanh,
                                     bias=bfc_sb[:, t:t + 1], scale=1.0)

        with tc.tile_pool(name="m_ps", bufs=1, space="PSUM") as m_ps, \
             tc.tile_pool(name="wm_sb", bufs=3) as wm_pool, \
             tc.tile_pool(name="out_sb", bufs=2) as out_pool:
            ps_m = [m_ps.tile([128, 512], f32, name=f"ps_m{k}", tag=f"ps_m{k}")
                    for k in range(8)]
            for t in range(FT):
                wm_t = wm_pool.tile([128, C], bf16, name="wm_t", tag="wm_t")
                nc.sync.dma_start(out=wm_t, in_=wm.ap()[ts(t, 128), :])
                for i in range(QT):
                    for n in range(C // 512):
                        nc.tensor.matmul(ps_m[i * 2 + n], gT[:, t, ts(i, 128)],
                                         wm_t[:, ds(512 * n, 512)],
                                         start=(t == 0), stop=(t == FT - 1))
            for i in range(QT):
                out_t = out_pool.tile([128, C], f32, name="out_t", tag="out_t")
                for n in range(C // 512):
                    nc.vector.tensor_add(out_t[:, ds(512 * n, 512)],
                                         ps_m[i * 2 + n],
                                         x2[:, i, ds(512 * n, 512)])
                    nc.vector.tensor_add(out_t[:, ds(512 * n, 512)],
                                         out_t[:, ds(512 * n, 512)],
                                         bm_bc[:, ds(512 * n, 512)])
                nc.scalar.dma_start(out=out.ap()[ts(i, 128), :], in_=out_t)


def _get_program(trivial_ln1, trivial_ln2, trivial_b):
    key = (trivial_ln1, trivial_ln2, trivial_b)
    if key not in _CACHED:
        _CACHED[key] = _build_program(trivial_ln1, trivial_ln2, trivial_b)
    return _CACHED[key]


def _tile_proj_weight(w):
    # [C, N] f32 -> [128, CCH, N] bf16 with partition = c % 128, chunk = c // 128
    w = np.asarray(w, np.float32).reshape(CCH, 128, -1)
    return np.ascontiguousarray(w.transpose(1, 0, 2).astype(ml_dtypes.bfloat16))


def _prep_in_maps(inputs):
    fl = lambda a: np.ascontiguousarray(np.asarray(a, np.float32))
    x = fl(inputs["x"])
    attn_w = fl(inputs["attn_w"])
    attn_b = fl(inputs["attn_b"])
    wf_full = fl(inputs["fc_w"])  # [C, F]
    # wf tiled: [FT, 128(c), CCH, 128(f')]
    wf_t = wf_full.reshape(CCH, 128, FT, 128).transpose(2, 1, 0, 3)
    wf_t = np.ascontiguousarray(wf_t.astype(ml_dtypes.bfloat16))
    pb = lambda b: np.ascontiguousarray(
        np.asarray(b, np.float32).reshape(-1, 128).T)  # [128, tiles]
    shared = {
        "wq": _tile_proj_weight(attn_w[:, 0:C]),
        "wk": _tile_proj_weight(attn_w[:, C:2 * C]),
        "wv": _tile_proj_weight(attn_w[:, 2 * C:3 * C]),
        "bq": pb(attn_b[0:C]), "bk": pb(attn_b[C:2 * C]),
        "bv": fl(attn_b[2 * C:3 * C]),
        "ln1w": fl(inputs["ln1_w"]), "ln1b": fl(inputs["ln1_b"]),
        "ln2w": fl(inputs["ln2_w"]), "ln2b": fl(inputs["ln2_b"]),
        "wp": _tile_proj_weight(inputs["attn_proj_w"]),
        "bp": fl(inputs["attn_proj_b"]),
        "wf": wf_t, "bf": pb(inputs["fc_b"]),
        "wm": np.ascontiguousarray(fl(inputs["mlp_proj_w"]).astype(ml_dtypes.bfloat16)),
        "bm": fl(inputs["mlp_proj_b"]),
    }
    in_maps = []
    for core in range(NCORES):
        b, r = core // GROUP, core % GROUP
        xb = np.roll(x[b], -TQ * r, axis=0)
        in_maps.append({"xf": np.ascontiguousarray(xb), **shared})
    return in_maps


def run(inputs, trace=False):
    trivial_ln1 = bool(np.all(np.asarray(inputs["ln1_w"]) == 1.0)
                       and np.all(np.asarray(inputs["ln1_b"]) == 0.0))
    trivial_ln2 = bool(np.all(np.asarray(inputs["ln2_w"]) == 1.0)
                       and np.all(np.asarray(inputs["ln2_b"]) == 0.0))
    trivial_b = bool(np.all(np.asarray(inputs["attn_b"]) == 0.0))
    nc = _get_program(trivial_ln1, trivial_ln2, trivial_b)
    in_maps = _prep_in_maps(inputs)
    res = run_bass_kernel_spmd(nc, in_maps, core_ids=list(range(NCORES)),
                               trace=trace)
    out = np.empty((B, T, C), np.float32)
    for core in range(NCORES):
        b, r = core // GROUP, core % GROUP
        out[b, TQ * r:TQ * (r + 1)] = res.results[core]["out"]
    return out, res


def kernel(**inputs):
    out, _ = run(inputs, trace=False)
    return out


# revision 20
# speedup vs baseline: 1.0414x; 1.0414x over previous
"""Trainium2 Bass kernel for a GPT-2 style transformer block (pre-LN, no mask).

Reference shapes: x [B=2, T=2048, C=1024], H=16 heads, MLP hidden 4C=4096.

Sharding (8 NeuronCores): data-parallel over B (cores 0-3 -> batch 0,
cores 4-7 -> batch 1); within each 4-core group the 2048 query rows are
split 512 per core. Every core redundantly computes K and V for its full
batch from a replicated (rotated) copy of x, so no collectives are needed:
attention rows and the MLP are fully local to a core. The per-core x is
rotated so that the core's own 512 query rows always sit at rows 0:512,
keeping the SPMD program identical across cores (softmax over the key
axis is permutation-invariant, so rotating the key order is harmless).

Compute layout: activations feeding matmul contractions are kept
feature-major ("transposed", [C, t]) via the DMA xbar transpose; scores
are computed as S^T = K Q^T per head ([tk, tq]) with two heads packed
into the 128-wide contraction via row tiling; exp runs on the scalar
engine straight out of PSUM; P @ V uses a [V | ones] stationary operand
so the softmax denominators accumulate in the same PSUM tile as Y^T.

Emission is interleaved for engine overlap: LayerNorm tiles alternate
with V matmuls; each head-pair's K^T/Q^T matmuls are followed
immediately by that pair's attention so the scalar engine's exp stream
overlaps the tensor engine's QKV work and the PE never idles long
enough for the HAM clock gate to re-throttle.

Weights are pre-tiled on the host into the exact SBUF layouts so every
DMA is a contiguous 128-partition transfer.
"""

import numpy as np
import ml_dtypes

import concourse.bass as bass
import concourse.bacc as bacc
import concourse.tile as tile
from concourse import mybir
from concourse.bass import ts, ds
from concourse.bass_utils import run_bass_kernel_spmd

f32 = mybir.dt.float32
bf16 = mybir.dt.bfloat16
AF = mybir.ActivationFunctionType
OP = mybir.AluOpType

B, T, C, H = 2, 2048, 1024, 16
DH = C // H          # 64
F = 4 * C            # 4096
NCORES = 8
GROUP = 4            # cores per batch
TQ = T // GROUP      # 512 query rows per core
NT = T // 128        # 16 token tiles
CCH = C // 128       # 8 contraction chunks over C
PAIRS = H // 2       # 8 head pairs
FT = F // 128        # 32 hidden tiles
QT = TQ // 128       # 4 own-row tiles

_CACHED = {}


def _bcast(ap, parts=128):
    """DRAM AP for a 1-D tensor broadcast across `parts` partitions."""
    return bass.AP(tensor=ap.tensor, offset=ap.offset, ap=[[0, parts]] + list(ap.ap))


def _build_program(trivial_ln1, trivial_ln2, trivial_b):
    nc = bacc.Bacc("TRN2", target_bir_lowering=False, debug=False,
                   num_devices=NCORES)

    xf = nc.dram_tensor("xf", [T, C], f32, kind="ExternalInput")
    # pre-tiled weights: [128 (c within chunk), CCH, out-features]
    wq = nc.dram_tensor("wq", [128, CCH, C], bf16, kind="ExternalInput")
    wk = nc.dram_tensor("wk", [128, CCH, C], bf16, kind="ExternalInput")
    wv = nc.dram_tensor("wv", [128, CCH, C], bf16, kind="ExternalInput")
    bqv = nc.dram_tensor("bq", [128, PAIRS], f32, kind="ExternalInput")
    bkv = nc.dram_tensor("bk", [128, PAIRS], f32, kind="ExternalInput")
    bvv = nc.dram_tensor("bv", [C], f32, kind="ExternalInput")
    ln1w = nc.dram_tensor("ln1w", [C], f32, kind="ExternalInput")
    ln1b = nc.dram_tensor("ln1b", [C], f32, kind="ExternalInput")
    ln2w = nc.dram_tensor("ln2w", [C], f32, kind="ExternalInput")
    ln2b = nc.dram_tensor("ln2b", [C], f32, kind="ExternalInput")
    wp = nc.dram_tensor("wp", [128, CCH, C], bf16, kind="ExternalInput")
    bp = nc.dram_tensor("bp", [C], f32, kind="ExternalInput")
    # wf pre-tiled per f'-tile: [FT, 128 (c), CCH, 128 (f')]
    wf = nc.dram_tensor("wf", [FT, 128, CCH, 128], bf16, kind="ExternalInput")
    bf_ = nc.dram_tensor("bf", [128, FT], f32, kind="ExternalInput")
    wm = nc.dram_tensor("wm", [F, C], bf16, kind="ExternalInput")
    bm = nc.dram_tensor("bm", [C], f32, kind="ExternalInput")
    out = nc.dram_tensor("out", [TQ, C], f32, kind="ExternalOutput")

    with tile.TileContext(nc) as tc:
        _emit(nc, tc, trivial_ln1, trivial_ln2, trivial_b,
              xf, wq, wk, wv, bqv, bkv, bvv, ln1w, ln1b, ln2w, ln2b,
              wp, bp, wf, bf_, wm, bm, out)
    nc.compile()
    return nc


def _emit(nc, tc, trivial_ln1, trivial_ln2, trivial_b,
          xf, wq, wk, wv, bqv, bkv, bvv, ln1w, ln1b, ln2w, ln2b,
          wp, bp, wf, bf_, wm, bm, out):
    from contextlib import ExitStack

    with ExitStack() as st:
        persist = st.enter_context(tc.tile_pool(name="persist", bufs=1))
        stat = st.enter_context(tc.tile_pool(name="stat", bufs=3))
        stream = st.enter_context(tc.tile_pool(name="stream", bufs=3))

        eps_t = persist.tile([128, 1], f32)
        nc.vector.memset(eps_t, 1e-5)

        def layer_norm(x_t, w_bc, b_bc, out_ap, trivial):
            """x_t [128, C] f32 -> out_ap [128, C] bf16 (normalized + affine)."""
            stats = stat.tile([128, 2, nc.vector.BN_STATS_DIM], f32, name="stats")
            nc.vector.bn_stats(out=stats[:, 0, :], in_=x_t[:, 0:512])
            nc.vector.bn_stats(out=stats[:, 1, :], in_=x_t[:, 512:1024])
            mv = stat.tile([128, nc.vector.BN_AGGR_DIM], f32, name="mv")
            nc.vector.bn_aggr(out=mv, in_=stats)
            rstd = stat.tile([128, 1], f32, name="rstd")
            nc.scalar.activation(rstd, mv[:, 1:2], AF.Sqrt, bias=eps_t)
            nc.vector.reciprocal(rstd, rstd)
            if trivial:
                nc.vector.tensor_scalar(out=out_ap, in0=x_t, scalar1=mv[:, 0:1],
                                        scalar2=rstd, op0=OP.subtract, op1=OP.mult)
            else:
                t1 = stat.tile([128, C], f32, name="t1", tag="ln_t1")
                nc.vector.tensor_scalar(out=t1, in0=x_t, scalar1=mv[:, 0:1],
                                        scalar2=rstd, op0=OP.subtract, op1=OP.mult)
                nc.vector.tensor_mul(t1, t1, w_bc)
                nc.vector.tensor_add(out_ap, t1, b_bc)

        # ---------------- pools (stack discipline per side) ----------------
        # left: pA = QKV-era (hT, wq, wk, ln1, biases); later pD = MLP-era
        # right: pR = v/ynT/wp/ln2 (through attn_proj); pB = wv (LN era only)
        stA = st.enter_context(ExitStack())
        pA = stA.enter_context(tc.tile_pool(name="pA", bufs=1, side="left"))
        pR = st.enter_context(tc.tile_pool(name="pR", bufs=1, side="right"))
        stB = st.enter_context(ExitStack())
        pB = stB.enter_context(tc.tile_pool(name="pB", bufs=1, side="right"))

        # wv + x stream first (the LN/V phase needs them immediately);
        # the q/k weights are emitted mid-loop so their DMA doesn't delay x
        wv_sb = pB.tile([128, CCH, C], bf16)
        nc.sync.dma_start(out=wv_sb, in_=wv.ap())
        if not trivial_b:
            bv_bc = pA.tile([128, C], f32)
            nc.sync.dma_start(out=bv_bc, in_=_bcast(bvv.ap()))
        else:
            bv_bc = None
        if not trivial_ln1:
            ln1w_bc = pA.tile([128, C], f32)
            nc.sync.dma_start(out=ln1w_bc, in_=_bcast(ln1w.ap()))
            ln1b_bc = pA.tile([128, C], f32)
            nc.sync.dma_start(out=ln1b_bc, in_=_bcast(ln1b.ap()))
        else:
            ln1w_bc = ln1b_bc = None

        hT = pA.tile([128, NT, CCH, 128], bf16)
        v_sb = pR.tile([128, NT, H, DH + 1], bf16)
        ynT = pR.tile([128, PAIRS, TQ], bf16)
        nc.vector.memset(v_sb[:, :, :, DH:DH + 1], 1.0)

        # ---- LN1 tiles interleaved with V matmuls ----
        with tc.tile_pool(name="v_ps", bufs=4, space="PSUM") as v_ps:
            for i in range(NT):
                x_t = stream.tile([128, C], f32, name="x_t", tag="x_t")
                nc.sync.dma_start(out=x_t, in_=xf.ap()[ts(i, 128), :])
                h_t = stream.tile([128, C], bf16, name="h_t", tag="h_t")
                layer_norm(x_t, ln1w_bc, ln1b_bc, h_t, trivial_ln1)
                nc.sync.dma_start_transpose(hT[:, i], h_t[:])
                if i == 2:
                    # q/k weights arrive while the LN/V pipeline runs
                    wq_sb = pA.tile([128, CCH, C], bf16)
                    nc.sync.dma_start(out=wq_sb, in_=wq.ap())
                    wk_sb = pA.tile([128, CCH, C], bf16)
                    nc.sync.dma_start(out=wk_sb, in_=wk.ap())
                    bq_sb = pA.tile([128, PAIRS], f32)
                    nc.sync.dma_start(out=bq_sb, in_=bqv.ap())
                    bk_sb = pA.tile([128, PAIRS], f32)
                    nc.sync.dma_start(out=bk_sb, in_=bkv.ap())
                ps0 = v_ps.tile([128, 512], f32, name="ps_v0", tag="ps_v")
                ps1 = v_ps.tile([128, 512], f32, name="ps_v1", tag="ps_v")
                pss = (ps0, ps1)
                for c in range(CCH):
                    for n in range(C // 512):
                        nc.tensor.matmul(pss[n], hT[:, i, c, :],
                                         wv_sb[:, c, ds(512 * n, 512)],
                                         start=(c == 0), stop=(c == CCH - 1))
                for n in range(C // 512):
                    if trivial_b:
                        nc.scalar.activation(v_sb[:, i, 8 * n:8 * n + 8, 0:DH],
                                             pss[n], AF.Identity)
                    else:
                        nc.vector.tensor_add(v_sb[:, i, 8 * n:8 * n + 8, 0:DH],
                                             pss[n], bv_bc[:, ds(512 * n, 512)])
        stB.close()

        # wp prefetch during attention (DMA is idle there)
        wp_sb = pR.tile([128, CCH, C], bf16)
        nc.sync.dma_start(out=wp_sb, in_=wp.ap())
        if not trivial_ln2:
            ln2w_bc = pR.tile([128, C], f32)
            nc.sync.dma_start(out=ln2w_bc, in_=_bcast(ln2w.ap()))
            ln2b_bc = pR.tile([128, C], f32)
            nc.sync.dma_start(out=ln2b_bc, in_=_bcast(ln2b.ap()))
        else:
            ln2w_bc = ln2b_bc = None
        bp_bc = pR.tile([128, C], f32)
        nc.sync.dma_start(out=bp_bc, in_=_bcast(bp.ap()))

        # ---- per-pair K^T/Q^T + attention, interleaved ----
        scale = 1.0 / float(np.sqrt(DH))
        with tc.tile_pool(name="kq_ps", bufs=2, space="PSUM") as kq_ps, \
             tc.tile_pool(name="s_ps", bufs=2, space="PSUM") as s_ps, \
             tc.tile_pool(name="y_ps", bufs=1, space="PSUM") as y_ps, \
             tc.tile_pool(name="kq_sb", bufs=2) as kq_sb, \
             tc.tile_pool(name="att_sb", bufs=3) as att_sb:
            for j in range(PAIRS):
                kT_j = kq_sb.tile([128, T], bf16, name="kT_j", tag="kT_j")
                for ng in range(T // 1024):
                    psn = [kq_ps.tile([128, 512], f32, name=f"ps_k{v}",
                                      tag="ps_kq") for v in range(2)]
                    for c in range(CCH):
                        for v in range(2):
                            n = 2 * ng + v
                            nc.tensor.matmul(psn[v], wk_sb[:, c, ts(j, 128)],
                                             hT[:, 4 * n:4 * n + 4, c, :],
                                             start=(c == 0), stop=(c == CCH - 1))
                    for v in range(2):
                        n = 2 * ng + v
                        nc.vector.tensor_scalar(out=kT_j[:, ds(512 * n, 512)],
                                                in0=psn[v], scalar1=bk_sb[:, j:j + 1],
                                                scalar2=None, op0=OP.add)
                qT_j = kq_sb.tile([128, TQ], bf16, name="qT_j", tag="qT_j")
                ps = kq_ps.tile([128, 512], f32, name="ps_q", tag="ps_kq")
                for c in range(CCH):
                    nc.tensor.matmul(ps, wq_sb[:, c, ts(j, 128)],
                                     hT[:, 0:QT, c, :],
                                     start=(c == 0), stop=(c == CCH - 1))
                nc.vector.tensor_scalar(out=qT_j, in0=ps,
                                        scalar1=bq_sb[:, j:j + 1],
                                        scalar2=None, op0=OP.add)

                ps_y1 = y_ps.tile([DH + 1, 512], f32, name="ps_y1", tag="ps_y1")
                ps_y2 = y_ps.tile([DH + 1, 512], f32, name="ps_y2", tag="ps_y2")
                for cidx in range(NT):
                    ps_s = s_ps.tile([128, 1024], f32, name="ps_s", tag="ps_s")
                    nc.tensor.matmul(ps_s[:, 0:512],
                                     kT_j[0:64, ts(cidx, 128)],
                                     qT_j[0:64, :], start=True, stop=True)
                    nc.tensor.matmul(ps_s[:, 512:1024],
                                     kT_j[64:128, ts(cidx, 128)],
                                     qT_j[64:128, :], start=True, stop=True,
                                     tile_position=(64, 0))
                    pT = att_sb.tile([128, 1024], bf16, name="pT", tag="pT")
                    nc.scalar.activation(pT, ps_s, AF.Exp, scale=scale)
                    nc.tensor.matmul(ps_y1, v_sb[:, cidx, 2 * j, :],
                                     pT[:, 0:512],
                                     start=(cidx == 0), stop=(cidx == NT - 1))
                    nc.tensor.matmul(ps_y2, v_sb[:, cidx, 2 * j + 1, :],
                                     pT[:, 512:1024],
                                     start=(cidx == 0), stop=(cidx == NT - 1))
                for u, ps_y in ((0, ps_y1), (1, ps_y2)):
                    # custom-DVE ops mis-read PSUM at a partition offset, so
                    # stage the sums row to SBUF partition 0 with a plain copy
                    rs0 = att_sb.tile([1, 512], f32, name="rs0", tag="rs0")
                    nc.vector.tensor_copy(rs0, ps_y[DH:DH + 1, :])
                    rs = att_sb.tile([1, 512], f32, name="rs", tag="rs")
                    nc.vector.reciprocal_approx_fast(rs, rs0)
                    bc = att_sb.tile([64, 512], f32, name="bc", tag="bc")
                    nc.gpsimd.partition_broadcast(bc, rs)
                    nc.vector.tensor_mul(ynT[64 * u:64 * u + 64, j, :],
                                         ps_y[0:DH, :], bc)
        stA.close()

        # ---- attn projection + residual + LN2 + h2^T ----
        pD = st.enter_context(tc.tile_pool(name="pD", bufs=1, side="left"))
        x2 = pD.tile([128, QT, C], f32)
        h2T = pD.tile([128, QT, CCH, 128], bf16)
        bfc_sb = pD.tile([128, FT], f32)
        nc.sync.dma_start(out=bfc_sb, in_=bf_.ap())
        bm_bc = pD.tile([128, C], f32)
        nc.sync.dma_start(out=bm_bc, in_=_bcast(bm.ap()))

        with tc.tile_pool(name="ap_ps", bufs=2, space="PSUM") as ap_ps:
            for i in range(QT):
                xb_t = stream.tile([128, C], f32, name="xb_t", tag="x_t")
                nc.sync.dma_start(out=xb_t, in_=xf.ap()[ts(i, 128), :])
                nc.vector.tensor_add(xb_t, xb_t, bp_bc)
                for n in range(C // 512):
                    ps = ap_ps.tile([128, 512], f32, name="ps_a", tag="ps_a")
                    for j in range(PAIRS):
                        nc.tensor.matmul(ps, ynT[:, j, ts(i, 128)],
                                         wp_sb[:, j, ds(512 * n, 512)],
                                         start=(j == 0), stop=(j == PAIRS - 1))
                    nc.vector.tensor_add(x2[:, i, ds(512 * n, 512)], ps,
                                         xb_t[:, ds(512 * n, 512)])
                h2_t = stream.tile([128, C], bf16, name="h2_t", tag="h_t")
                layer_norm(x2[:, i, :], ln2w_bc, ln2b_bc, h2_t, trivial_ln2)
                nc.sync.dma_start_transpose(h2T[:, i], h2_t[:])

        # ---- MLP ----
        gT = pD.tile([128, FT, TQ], bf16)
        with tc.tile_pool(name="fc_ps", bufs=4, space="PSUM") as fc_ps, \
             tc.tile_pool(name="wf_sb", bufs=4) as wf_pool:
            for t in range(FT):
                wf_t = wf_pool.tile([128, CCH, 128], bf16, name="wf_t", tag="wf_t")
                nc.sync.dma_start(out=wf_t, in_=wf.ap()[t])
                ps = fc_ps.tile([128, 512], f32, name="ps_f", tag="ps_f")
                for c in range(CCH):
                    nc.tensor.matmul(ps, wf_t[:, c, :], h2T[:, 0:QT, c, :],
                                     start=(c == 0), stop=(c == CCH - 1))
                nc.scalar.activation(gT[:, t, :], ps, AF.Gelu_apprx_tanh,
                                     bias=bfc_sb[:, t:t + 1], scale=1.0)

        with tc.tile_pool(name="m_ps", bufs=1, space="PSUM") as m_ps, \
             tc.tile_pool(name="wm_sb", bufs=3) as wm_pool, \
             tc.tile_pool(name="out_sb", bufs=2) as out_pool:
            ps_m = [m_ps.tile([128, 512], f32, name=f"ps_m{k}", tag=f"ps_m{k}")
                    for k in range(8)]
            for t in range(FT):
                wm_t = wm_pool.tile([128, C], bf16, name="wm_t", tag="wm_t")
                nc.sync.dma_start(out=wm_t, in_=wm.ap()[ts(t, 128), :])
                for i in range(QT):
                    for n in range(C // 512):
                        nc.tensor.matmul(ps_m[i * 2 + n], gT[:, t, ts(i, 128)],
                                         wm_t[:, ds(512 * n, 512)],
                                         start=(t == 0), stop=(t == FT - 1))
            for i in range(QT):
                out_t = out_pool.tile([128, C], f32, name="out_t", tag="out_t")
                for n in range(C // 512):
                    nc.vector.tensor_add(out_t[:, ds(512 * n, 512)],
                                         ps_m[i * 2 + n],
                                         x2[:, i, ds(512 * n, 512)])
                    nc.vector.tensor_add(out_t[:, ds(512 * n, 512)],
                                         out_t[:, ds(512 * n, 512)],
                                         bm_bc[:, ds(512 * n, 512)])
                nc.sync.dma_start(out=out.ap()[ts(i, 128), :], in_=out_t)


def _get_program(trivial_ln1, trivial_ln2, trivial_b):
    key = (trivial_ln1, trivial_ln2, trivial_b)
    if key not in _CACHED:
        _CACHED[key] = _build_program(trivial_ln1, trivial_ln2, trivial_b)
    return _CACHED[key]


def _tile_proj_weight(w):
    # [C, N] f32 -> [128, CCH, N] bf16 with partition = c % 128, chunk = c // 128
    w = np.asarray(w, np.float32).reshape(CCH, 128, -1)
    return np.ascontiguousarray(w.transpose(1, 0, 2).astype(ml_dtypes.bfloat16))


def _prep_in_maps(inputs):
    fl = lambda a: np.ascontiguousarray(np.asarray(a, np.float32))
    x = fl(inputs["x"])
    attn_w = fl(inputs["attn_w"])
    attn_b = fl(inputs["attn_b"])
    wf_full = fl(inputs["fc_w"])  # [C, F]
    # wf tiled: [FT, 128(c), CCH, 128(f')]
    wf_t = wf_full.reshape(CCH, 128, FT, 128).transpose(2, 1, 0, 3)
    wf_t = np.ascontiguousarray(wf_t.astype(ml_dtypes.bfloat16))
    pb = lambda b: np.ascontiguousarray(
        np.asarray(b, np.float32).reshape(-1, 128).T)  # [128, tiles]
    shared = {
        "wq": _tile_proj_weight(attn_w[:, 0:C]),
        "wk": _tile_proj_weight(attn_w[:, C:2 * C]),
        "wv": _tile_proj_weight(attn_w[:, 2 * C:3 * C]),
        "bq": pb(attn_b[0:C]), "bk": pb(attn_b[C:2 * C]),
        "bv": fl(attn_b[2 * C:3 * C]),
        "ln1w": fl(inputs["ln1_w"]), "ln1b": fl(inputs["ln1_b"]),
        "ln2w": fl(inputs["ln2_w"]), "ln2b": fl(inputs["ln2_b"]),
        "wp": _tile_proj_weight(inputs["attn_proj_w"]),
        "bp": fl(inputs["attn_proj_b"]),
        "wf": wf_t, "bf": pb(inputs["fc_b"]),
        "wm": np.ascontiguousarray(fl(inputs["mlp_proj_w"]).astype(ml_dtypes.bfloat16)),
        "bm": fl(inputs["mlp_proj_b"]),
    }
    in_maps = []
    for core in range(NCORES):
        b, r = core // GROUP, core % GROUP
        xb = np.roll(x[b], -TQ * r, axis=0)
        in_maps.append({"xf": np.ascontiguousarray(xb), **shared})
    return in_maps


def run(inputs, trace=False):
    trivial_ln1 = bool(np.all(np.asarray(inputs["ln1_w"]) == 1.0)
                       and np.all(np.asarray(inputs["ln1_b"]) == 0.0))
    trivial_ln2 = bool(np.all(np.asarray(inputs["ln2_w"]) == 1.0)
                       and np.all(np.asarray(inputs["ln2_b"]) == 0.0))
    trivial_b = bool(np.all(np.asarray(inputs["attn_b"]) == 0.0))
    nc = _get_program(trivial_ln1, trivial_ln2, trivial_b)
    in_maps = _prep_in_maps(inputs)
    res = run_bass_kernel_spmd(nc, in_maps, core_ids=list(range(NCORES)),
                               trace=trace)
    out = np.empty((B, T, C), np.float32)
    for core in range(NCORES):
        b, r = core // GROUP, core % GROUP
        out[b, TQ * r:TQ * (r + 1)] = res.results[core]["out"]
    return out, res


def kernel(**inputs):
    out, _ = run(inputs, trace=False)
    return out
